# revision 7
# baseline (speedup 1.0000x reference)
"""Causal self-attention (B=4, T=2048, C=512, H=8, D=64) on 8 TRN2 NeuronCores.

Sharding: core = (batch b, head-group hg) with 4 batches x 2 head groups of 4
heads.  Each core computes q/k/v projections for its 4 heads, causal
attention, and a partial output projection (its 256 rows of W_out); the host
sums the two head-group partials per batch.

Per-core kernel layout notes:
  - x is fed pre-transposed and kc-chunked ([KC, 128, T]) and weights are fed
    pre-tiled ([128, KC, M]) so every DMA is a plain 2D strided transfer
    (hardware descriptor-gen path, no gather).
  - Attention computes S^T blocks ([tk, tq]) directly by swapping matmul
    operands, so no on-chip transposes are needed anywhere.  tq widths are
    ragged (only tq >= tk is computed); a triu mask handles diagonal blocks.
  - Softmax denominators come from a ones-column appended to V (row 64 of the
    PV accumulator); normalization multiplies the O^T eviction by a
    partition-broadcast reciprocal row.
  - The scalar engine runs ONLY the exp activations (it is the second
    bottleneck after the PE); all PSUM evictions go to vector/gpsimd.
  - Background PE work (next supertile's projections, deferred output
    projections) is paced into the attention stream to fill the gap left by
    exp latency; deferred y-projections are biased toward the last supertile
    where no projection work remains.
  - Matmul operands are bf16 (inputs rounded host-side); accumulation and the
    softmax arithmetic stay fp32 in PSUM.  y partials are written bf16 and
    summed in fp32 on the host.
"""

import os
from contextlib import ExitStack

import numpy as np
import ml_dtypes

import concourse.bass as bass
import concourse.tile as tile
from concourse import bacc, mybir
from concourse.bass import ts, ds
from concourse.bass_utils import run_bass_kernel_spmd
from concourse.masks import make_upper_triangular

# Problem constants (hardcoded per harness contract).
B = 4
T = 2048
C = 512
H = 8
D = 64
HG = 2                 # head groups (tensor-parallel dim)
HPC = H // HG          # heads per core = 4
M = HPC * D            # local head width = 256
P = 128
NT = T // P            # 16 t-tiles
NS = T // 512          # 4 t-supertiles
KC = C // P            # 4 contraction chunks of x
F32 = mybir.dt.float32
BF16 = mybir.dt.bfloat16

_LAST_RESULTS = None   # stashed BassKernelResults for test harness inspection


def build_attention_kernel():
    nc = bacc.Bacc("TRN2", target_bir_lowering=False, debug=False, num_devices=B * HG)

    xT = nc.dram_tensor("xT", [KC, P, T], BF16, kind="ExternalInput").ap()
    wq = nc.dram_tensor("wq", [P, KC, M], BF16, kind="ExternalInput").ap()
    wk = nc.dram_tensor("wk", [P, KC, M], BF16, kind="ExternalInput").ap()
    wv = nc.dram_tensor("wv", [P, KC, M], BF16, kind="ExternalInput").ap()
    wo = nc.dram_tensor("wo", [P, M // P, C], BF16, kind="ExternalInput").ap()
    y = nc.dram_tensor("y", [T, C], BF16, kind="ExternalOutput").ap()

    with tile.TileContext(nc) as tc:
        with ExitStack() as ctx:
            emit_kernel(ctx, tc, xT, wq, wk, wv, wo, y)
    nc.compile()
    return nc


def emit_kernel(ctx, tc, xT, wq, wk, wv, wo, y):
    nc = tc.nc
    Exp = mybir.ActivationFunctionType.Exp
    scale = 1.0 / np.sqrt(D)

    const = ctx.enter_context(tc.tile_pool(name="const", bufs=1))
    xt_pool = ctx.enter_context(tc.tile_pool(name="xt", bufs=1))
    w_pool = ctx.enter_context(tc.tile_pool(name="w", bufs=1))
    qkv_pool = ctx.enter_context(tc.tile_pool(name="qkv", bufs=1))
    pt_pool = ctx.enter_context(tc.tile_pool(name="pt", bufs=8))
    ot_pool = ctx.enter_context(tc.tile_pool(name="ot", bufs=1))
    ysb_pool = ctx.enter_context(tc.tile_pool(name="ysb", bufs=3))
    small_pool = ctx.enter_context(tc.tile_pool(name="small", bufs=6))
    psum_s = ctx.enter_context(tc.tile_pool(name="psum_s", bufs=2, space="PSUM"))
    psum_ot = ctx.enter_context(tc.tile_pool(name="psum_ot", bufs=2, space="PSUM"))

    # --- constants ---
    triu_f32 = const.tile([P, P], F32)
    make_upper_triangular(nc, triu_f32[:], val=1.0, diag=True)
    triu = const.tile([P, P], BF16)
    nc.vector.tensor_copy(triu[:], triu_f32[:])

    # --- PE clock pre-warm: dummy back-to-back matmuls during the initial
    # DMA wait flip the HAM clock gate to full rate before real work ---
    warm_in = const.tile([P, D], BF16)
    nc.gpsimd.memset(warm_in[:], 1.0)
    warm_ps = psum_s.tile([P, 2, 512], F32, name="s_ps")
    for i in range(56):
        nc.tensor.matmul(
            warm_ps[0:D, 0, 0:D], warm_in[:], warm_in[:], start=True, stop=True
        )

    # --- load weights and xT across four DMA queues; the first projection
    # chunk's inputs (wq, wk, xt tc0) land first ---
    wq_sb = w_pool.tile([P, KC, M], BF16)
    nc.sync.dma_start(wq_sb[:], wq)
    wk_sb = w_pool.tile([P, KC, M], BF16)
    nc.gpsimd.dma_start(wk_sb[:], wk)

    xt_sb = []
    for kc in range(KC):
        t_ = xt_pool.tile([P, T], BF16, name=f"xt{kc}")
        xt_sb.append(t_)

    xt_q = [nc.sync, nc.gpsimd, nc.scalar, nc.scalar]

    def load_xt(tc_):
        for kc in range(KC):
            xt_q[kc].dma_start(
                xt_sb[kc][:, ts(tc_, 512)], xT[kc, :, ts(tc_, 512)]
            )

    load_xt(0)
    wv_sb = w_pool.tile([P, KC, M], BF16)
    nc.sync.dma_start(wv_sb[:], wv)
    wo_sb = w_pool.tile([P, M // P, C], BF16)
    nc.gpsimd.dma_start(wo_sb[:], wo)
    for tc_ in range(1, NS):
        load_xt(tc_)

    # --- PSUM evictions: only vector/scalar can read PSUM; keep the scalar
    # engine mostly exp-only (it only takes the off-critical-path yproj
    # evictions, alternating) ---
    _ev_flip = [0]

    def evict(dst, src, alternate=False):
        if alternate:
            _ev_flip[0] ^= 1
            if _ev_flip[0]:
                nc.scalar.copy(dst, src)
                return
        nc.vector.tensor_copy(dst, src)

    # --- QKV projection emitters ---
    qt_sb = [qkv_pool.tile([P, T], BF16, name=f"qt{i}") for i in range(M // P)]
    kt_sb = [qkv_pool.tile([P, T], BF16, name=f"kt{i}") for i in range(M // P)]
    # V (+ones col): [128, NT, HPC, D+1]; V block tt rows t in tile, per head.
    v_sb = qkv_pool.tile([P, NT, HPC, D + 1], BF16)
    nc.gpsimd.memset(v_sb[:, :, :, D : D + 1], 1.0)

    def emit_qk_group(tc_, mo, w_sb, dst):
        s_ps = psum_s.tile([P, 2, 512], F32, name="s_ps")
        ps = s_ps[:, 0, :]
        for kc in range(KC):
            nc.tensor.matmul(
                ps,
                w_sb[:, kc, ts(mo, P)],
                xt_sb[kc][:, ts(tc_, 512)],
                start=(kc == 0),
                stop=(kc == KC - 1),
            )
        evict(dst[mo][:, ts(tc_, 512)], ps)

    def emit_v_block(tt):
        s_ps = psum_s.tile([P, 2, 512], F32, name="s_ps")
        ps = s_ps[:, 0, 0:M]
        for kc in range(KC):
            nc.tensor.matmul(
                ps,
                xt_sb[kc][:, ts(tt, P)],
                wv_sb[:, kc, :],
                start=(kc == 0),
                stop=(kc == KC - 1),
            )
        evict(v_sb[:, tt, :, 0:D], ps.rearrange("p (h d) -> p h d", d=D))

    # --- output projection (deferred into later supertiles' PE streams) ---
    ot_sb = [ot_pool.tile([P, T], BF16, name=f"ot{i}") for i in range(M // P)]
    _ydma_q = [nc.sync, nc.gpsimd, nc.scalar]
    _ydma_flip = [0]

    def emit_yproj(tt):
        s_ps = psum_s.tile([P, 2, 512], F32, name="s_ps")
        ps = s_ps[:, 0, :]
        for mo in range(M // P):
            nc.tensor.matmul(
                ps,
                ot_sb[mo][:, ts(tt, P)],
                wo_sb[:, mo, :],
                start=(mo == 0),
                stop=(mo == M // P - 1),
            )
        y_sb = ysb_pool.tile([P, C], BF16)
        evict(y_sb[:], ps, alternate=True)
        q = _ydma_q[_ydma_flip[0] % 3]
        _ydma_flip[0] += 1
        q.dma_start(y[ts(tt, P), :], y_sb[:])

    # --- softmax normalization chain (hs-split for low tail latency) ---
    def emit_norm(s, mo, ot_ps):
        sums = small_pool.tile([1, 2, 512], F32)
        recip = small_pool.tile([1, 2, 512], F32)
        for hs in (0, 1):
            nc.vector.tensor_copy(sums[:, hs, :], ot_ps[ds(D, 1), hs, :])
            nc.vector.reciprocal_approx_fast(recip[:, hs, :], sums[:, hs, :])
        bcast = small_pool.tile([D, 2, 512], F32)
        for hs in (0, 1):
            nc.gpsimd.partition_broadcast(bcast[:, hs, :], recip[:, hs, :])
        for hs, po in ((0, 0), (1, D)):
            nc.vector.tensor_mul(
                ot_sb[mo][ds(po, D), ts(s, 512)],
                ot_ps[0:D, hs, :],
                bcast[:, hs, :],
            )

    # --- attention block: S^T -> exp -> (mask) -> PV for one (s, j, mo) ---
    def emit_attn(s, j, mo, ot_units, nblk, bg, bg_state):
        off = max(0, j - 4 * s) * P
        n = 512 - off
        s_ps = psum_s.tile([P, 2, 512], F32, name="s_ps")
        for hs, po in ((0, 0), (1, D)):
            nc.tensor.matmul(
                s_ps[:, hs, 0:n],
                kt_sb[mo][ds(po, D), ts(j, P)],
                qt_sb[mo][ds(po, D), ds(512 * s + off, n)],
                start=True,
                stop=True,
            )
        pt = pt_pool.tile([P, 2, 512], BF16)
        nc.scalar.activation(pt[:, :, 0:n], s_ps[:, :, 0:n], Exp, scale=scale)
        # background PE work lands between S^T and PV so the PE chews on it
        # while the exp runs
        pop_bg(bg, bg_state)
        if off > 0 or j == 4 * s:
            # first 128 cols of the ragged region are the diagonal block
            for hs in (0, 1):
                nc.vector.tensor_mul(pt[:, hs, 0:P], pt[:, hs, 0:P], triu[:])
        for hs, h in ((0, 2 * mo), (1, 2 * mo + 1)):
            nc.tensor.matmul(
                ot_units[mo][:, hs, ds(off, n)],
                v_sb[:, j, h, :],
                pt[:, hs, ds(0, n)],
                start=(j == 0),
                stop=(j == nblk - 1),
            )

    # --- background-work pacing: drain bg evenly across a window's slots ---
    def pop_bg(bg, bg_state):
        bg_state[0] += 1
        target = (len(bg[1]) * bg_state[0] + bg_state[1] - 1) // bg_state[1]
        while bg[0] < min(target, len(bg[1])):
            bg[1][bg[0]]()
            bg[0] += 1

    def drain_bg(bg):
        while bg[0] < len(bg[1]):
            bg[1][bg[0]]()
            bg[0] += 1

    def proj_groups_for(tc_):
        groups = []
        for mo in range(M // P):
            for w_sb, dst in ((wq_sb, qt_sb), (wk_sb, kt_sb)):
                groups.append(lambda t=tc_, m=mo, w=w_sb, d=dst: emit_qk_group(t, m, w, d))
        for tt in range(4 * tc_, 4 * tc_ + 4):
            groups.append(lambda t=tt: emit_v_block(t))
        return groups

    # minimal prefix before attention can start: q/k for both head pairs of
    # supertile 0 plus the first V block
    g0 = proj_groups_for(0)
    for g in g0[:4] + [g0[4]]:
        g()
    rest0 = g0[5:]           # v blocks tt1..tt3

    # deferred y-projection schedule: each supertile's 4 tiles become
    # available after its normalization; we push most of them into the last
    # supertile's stream where no projection work remains.
    yproj_carry = []
    carry_cap = {1: 2, 2: 3, 3: 99}

    for s in range(NS):
        nblk = 4 * (s + 1)
        bg_list = []
        if s == 0:
            bg_list += rest0
        if s + 1 < NS:
            bg_list += proj_groups_for(s + 1)
        ncarry = min(carry_cap.get(s, 0), len(yproj_carry))
        for tt in yproj_carry[:ncarry]:
            bg_list.append(lambda t=tt: emit_yproj(t))
        yproj_carry = yproj_carry[ncarry:]

        ot_units = [
            psum_ot.tile([D + 1, 2, 512], F32, name="ot_ps")
            for _ in range(M // P)
        ]

        if s + 1 < NS:
            # mo-inner: both head pairs advance together
            nslots = nblk * 2
            bg = [0, bg_list]
            bg_state = [0, nslots]
            for j in range(nblk):
                for mo in range(M // P):
                    emit_attn(s, j, mo, ot_units, nblk, bg, bg_state)
            drain_bg(bg)
            for mo in range(M // P):
                emit_norm(s, mo, ot_units[mo])
        else:
            # final supertile: mo-outer so head pair 0 finishes (and
            # normalizes) while head pair 1 is still streaming
            half = (len(bg_list) + 1) // 2
            bgA = [0, bg_list[:half]]
            bgB = [0, bg_list[half:]]
            bg_state_a = [0, nblk]
            for j in range(nblk):
                emit_attn(s, j, 0, ot_units, nblk, bgA, bg_state_a)
            drain_bg(bgA)
            emit_norm(s, 0, ot_units[0])
            bg_state_b = [0, nblk]
            for j in range(nblk):
                emit_attn(s, j, 1, ot_units, nblk, bgB, bg_state_b)
            drain_bg(bgB)

            # tail: overlap the mo=1 norm chain with the mo=0 halves of the
            # final output projections, then finish and store
            tts = list(range(4 * s, 4 * s + 4))
            yp = [
                psum_s.tile([P, 2, 512], F32, name="s_ps"),
                psum_s.tile([P, 2, 512], F32, name="s_ps"),
            ]
            for i, tt in enumerate(tts):
                nc.tensor.matmul(
                    yp[i // 2][:, i % 2, :],
                    ot_sb[0][:, ts(tt, P)],
                    wo_sb[:, 0, :],
                    start=True,
                    stop=False,
                )
            emit_norm(s, 1, ot_units[1])
            for i, tt in enumerate(tts):
                nc.tensor.matmul(
                    yp[i // 2][:, i % 2, :],
                    ot_sb[1][:, ts(tt, P)],
                    wo_sb[:, 1, :],
                    start=False,
                    stop=True,
                )
                y_sb = ysb_pool.tile([P, C], BF16)
                evict(y_sb[:], yp[i // 2][:, i % 2, :])
                _ydma_q[i % 3].dma_start(y[ts(tt, P), :], y_sb[:])

        yproj_carry += list(range(4 * s, 4 * s + 4)) if s + 1 < NS else []


def shard_inputs(x, W_qkv, W_out):
    """Full inputs -> list of 8 per-core input dicts (core = b*HG + hg)."""
    bf16 = ml_dtypes.bfloat16
    x = np.asarray(x, dtype=np.float32)
    W_qkv = np.asarray(W_qkv, dtype=np.float32).astype(bf16)
    W_out = np.asarray(W_out, dtype=np.float32).astype(bf16)

    def tile_w(w):  # [C, M] -> [P, KC, M]
        return np.ascontiguousarray(
            w.reshape(KC, P, -1).transpose(1, 0, 2)
        )

    in_maps = []
    for b in range(B):
        xT = np.ascontiguousarray(
            x[b].T.astype(bf16).reshape(KC, P, T)
        )
        for hg in range(HG):
            cols = slice(hg * M, (hg + 1) * M)
            wo_full = W_out[hg * M : (hg + 1) * M, :]     # [M, C]
            in_maps.append(
                {
                    "xT": xT,
                    "wq": tile_w(W_qkv[:, 0 * C :][:, cols]),
                    "wk": tile_w(W_qkv[:, 1 * C :][:, cols]),
                    "wv": tile_w(W_qkv[:, 2 * C :][:, cols]),
                    "wo": np.ascontiguousarray(
                        wo_full.reshape(M // P, P, C).transpose(1, 0, 2)
                    ),
                }
            )
    return in_maps


_NC_CACHE = None


def kernel(x, W_qkv, W_out):
    global _NC_CACHE, _LAST_RESULTS
    if _NC_CACHE is None:
        _NC_CACHE = build_attention_kernel()
    nc = _NC_CACHE
    in_maps = shard_inputs(x, W_qkv, W_out)
    kwargs = {}
    if os.environ.get("BASS_KERNEL_TRACE"):
        kwargs = dict(trace=True, tmpdir=os.environ.get("BASS_KERNEL_TRACE_DIR"))
    res = run_bass_kernel_spmd(nc, in_maps, core_ids=list(range(B * HG)), **kwargs)
    _LAST_RESULTS = res
    out = np.empty((B, T, C), dtype=np.float32)
    for b in range(B):
        out[b] = (
            res.results[b * HG]["y"].astype(np.float32)
            + res.results[b * HG + 1]["y"].astype(np.float32)
        )
    return out


# revision 13
# speedup vs baseline: 1.0529x; 1.0529x over previous
"""Causal self-attention (B=4, T=2048, C=512, H=8, D=64) on 8 TRN2 NeuronCores.

Sharding: core = (batch b, head-group hg) with 4 batches x 2 head groups of 4
heads.  Each core computes q/k/v projections for its 4 heads, causal
attention, and a partial output projection (its 256 rows of W_out); the host
sums the two head-group partials per batch.

Per-core kernel layout notes:
  - x is fed pre-transposed and kc-chunked ([KC, 128, T]) and weights are fed
    pre-tiled ([128, KC, M]) so every DMA is a plain 2D strided transfer
    (hardware descriptor-gen path, no gather).
  - Attention computes S^T blocks ([tk, tq]) directly by swapping matmul
    operands, so no on-chip transposes are needed anywhere.  tq widths are
    ragged (only tq >= tk is computed); a triu mask handles diagonal blocks.
  - Softmax denominators come from a ones-column appended to V (row 64 of the
    PV accumulator); normalization multiplies the O^T eviction by a
    partition-broadcast reciprocal row.
  - The scalar engine runs ONLY the exp activations (it is the second
    bottleneck after the PE); all PSUM evictions go to vector/gpsimd.
  - Background PE work (next supertile's projections, deferred output
    projections) is paced into the attention stream to fill the gap left by
    exp latency; deferred y-projections are biased toward the last supertile
    where no projection work remains.
  - Matmul operands are bf16 (inputs rounded host-side); accumulation and the
    softmax arithmetic stay fp32 in PSUM.  y partials are written bf16 and
    summed in fp32 on the host.
"""

import os
from contextlib import ExitStack

import numpy as np
import ml_dtypes

import concourse.bass as bass
import concourse.tile as tile
from concourse import bacc, mybir
from concourse.bass import ts, ds
from concourse.bass_utils import run_bass_kernel_spmd
from concourse.masks import make_upper_triangular

# Problem constants (hardcoded per harness contract).
B = 4
T = 2048
C = 512
H = 8
D = 64
HG = 2                 # head groups (tensor-parallel dim)
HPC = H // HG          # heads per core = 4
M = HPC * D            # local head width = 256
P = 128
NT = T // P            # 16 t-tiles
NS = T // 512          # 4 t-supertiles
KC = C // P            # 4 contraction chunks of x
F32 = mybir.dt.float32
BF16 = mybir.dt.bfloat16

_LAST_RESULTS = None   # stashed BassKernelResults for test harness inspection


def build_attention_kernel():
    nc = bacc.Bacc("TRN2", target_bir_lowering=False, debug=False, num_devices=B * HG)

    xT = nc.dram_tensor("xT", [KC, P, T], BF16, kind="ExternalInput").ap()
    wq = nc.dram_tensor("wq", [P, KC, M], BF16, kind="ExternalInput").ap()
    wk = nc.dram_tensor("wk", [P, KC, M], BF16, kind="ExternalInput").ap()
    wv = nc.dram_tensor("wv", [P, KC, M], BF16, kind="ExternalInput").ap()
    wo = nc.dram_tensor("wo", [P, M // P, C], BF16, kind="ExternalInput").ap()
    y = nc.dram_tensor("y", [T, C], BF16, kind="ExternalOutput").ap()

    with tile.TileContext(nc) as tc:
        with ExitStack() as ctx:
            emit_kernel(ctx, tc, xT, wq, wk, wv, wo, y)
    nc.compile()
    return nc


def emit_kernel(ctx, tc, xT, wq, wk, wv, wo, y):
    nc = tc.nc
    Exp = mybir.ActivationFunctionType.Exp
    scale = 1.0 / np.sqrt(D)

    const = ctx.enter_context(tc.tile_pool(name="const", bufs=1))
    xt_pool = ctx.enter_context(tc.tile_pool(name="xt", bufs=1))
    w_pool = ctx.enter_context(tc.tile_pool(name="w", bufs=1))
    qkv_pool = ctx.enter_context(tc.tile_pool(name="qkv", bufs=1))
    pt_pool = ctx.enter_context(tc.tile_pool(name="pt", bufs=8))
    ot_pool = ctx.enter_context(tc.tile_pool(name="ot", bufs=1))
    ysb_pool = ctx.enter_context(tc.tile_pool(name="ysb", bufs=3))
    small_pool = ctx.enter_context(tc.tile_pool(name="small", bufs=6))
    psum_s = ctx.enter_context(tc.tile_pool(name="psum_s", bufs=2, space="PSUM"))
    psum_ot = ctx.enter_context(tc.tile_pool(name="psum_ot", bufs=2, space="PSUM"))

    # --- constants ---
    triu_f32 = const.tile([P, P], F32)
    make_upper_triangular(nc, triu_f32[:], val=1.0, diag=True)
    triu = const.tile([P, P], BF16)
    nc.vector.tensor_copy(triu[:], triu_f32[:])

    # --- PE clock pre-warm: dummy back-to-back matmuls during the initial
    # DMA wait flip the HAM clock gate to full rate before real work ---
    warm_in = const.tile([P, D], BF16)
    nc.gpsimd.memset(warm_in[:], 1.0)
    warm_ps = psum_s.tile([P, 2, 512], F32, name="s_ps")
    for i in range(56):
        nc.tensor.matmul(
            warm_ps[0:D, 0, 0:D], warm_in[:], warm_in[:], start=True, stop=True
        )

    # --- load weights and xT across four DMA queues; the first projection
    # chunk's inputs (wq, wk, xt tc0) land first ---
    wq_sb = w_pool.tile([P, KC, M], BF16)
    nc.sync.dma_start(wq_sb[:], wq)
    wk_sb = w_pool.tile([P, KC, M], BF16)
    nc.gpsimd.dma_start(wk_sb[:], wk)

    xt_sb = []
    for kc in range(KC):
        t_ = xt_pool.tile([P, T], BF16, name=f"xt{kc}")
        xt_sb.append(t_)

    xt_q = [nc.sync, nc.gpsimd, nc.sync, nc.gpsimd]

    def load_xt(tc_):
        for kc in range(KC):
            xt_q[kc].dma_start(
                xt_sb[kc][:, ts(tc_, 512)], xT[kc, :, ts(tc_, 512)]
            )

    load_xt(0)
    wv_sb = w_pool.tile([P, KC, M], BF16)
    nc.sync.dma_start(wv_sb[:], wv)
    wo_sb = w_pool.tile([P, M // P, C], BF16)
    nc.gpsimd.dma_start(wo_sb[:], wo)
    for tc_ in range(1, NS):
        load_xt(tc_)

    # --- PSUM evictions: only vector/scalar can read PSUM; the scalar
    # engine is kept exp-only so everything lands on vector ---
    def evict(dst, src, alternate=False):
        nc.vector.tensor_copy(dst, src)

    # --- QKV projection emitters ---
    qt_sb = [qkv_pool.tile([P, T], BF16, name=f"qt{i}") for i in range(M // P)]
    kt_sb = [qkv_pool.tile([P, T], BF16, name=f"kt{i}") for i in range(M // P)]
    # V (+ones col): [128, NT, HPC, D+1]; V block tt rows t in tile, per head.
    v_sb = qkv_pool.tile([P, NT, HPC, D + 1], BF16)
    nc.gpsimd.memset(v_sb[:, :, :, D : D + 1], 1.0)

    def emit_qk_group(tc_, mo, w_sb, dst):
        s_ps = psum_s.tile([P, 2, 512], F32, name="s_ps")
        ps = s_ps[:, 0, :]
        for kc in range(KC):
            nc.tensor.matmul(
                ps,
                w_sb[:, kc, ts(mo, P)],
                xt_sb[kc][:, ts(tc_, 512)],
                start=(kc == 0),
                stop=(kc == KC - 1),
            )
        evict(dst[mo][:, ts(tc_, 512)], ps)

    def emit_v_block(tt):
        s_ps = psum_s.tile([P, 2, 512], F32, name="s_ps")
        ps = s_ps[:, 0, 0:M]
        for kc in range(KC):
            nc.tensor.matmul(
                ps,
                xt_sb[kc][:, ts(tt, P)],
                wv_sb[:, kc, :],
                start=(kc == 0),
                stop=(kc == KC - 1),
            )
        evict(v_sb[:, tt, :, 0:D], ps.rearrange("p (h d) -> p h d", d=D))

    # --- output projection (deferred into later supertiles' PE streams) ---
    ot_sb = [ot_pool.tile([P, T], BF16, name=f"ot{i}") for i in range(M // P)]
    _ydma_q = [nc.sync, nc.gpsimd]
    _ydma_flip = [0]

    def emit_yproj(tt):
        s_ps = psum_s.tile([P, 2, 512], F32, name="s_ps")
        ps = s_ps[:, 0, :]
        for mo in range(M // P):
            nc.tensor.matmul(
                ps,
                ot_sb[mo][:, ts(tt, P)],
                wo_sb[:, mo, :],
                start=(mo == 0),
                stop=(mo == M // P - 1),
            )
        y_sb = ysb_pool.tile([P, C], BF16)
        evict(y_sb[:], ps)
        q = _ydma_q[_ydma_flip[0] % 2]
        _ydma_flip[0] += 1
        q.dma_start(y[ts(tt, P), :], y_sb[:])

    # --- softmax normalization chain (hs-split for low tail latency) ---
    def emit_norm(s, mo, ot_ps):
        sums = small_pool.tile([1, 2, 512], F32)
        recip = small_pool.tile([1, 2, 512], F32)
        for hs in (0, 1):
            nc.vector.tensor_copy(sums[:, hs, :], ot_ps[ds(D, 1), hs, :])
            nc.vector.reciprocal_approx_fast(recip[:, hs, :], sums[:, hs, :])
        bcast = small_pool.tile([D, 2, 512], F32)
        for hs in (0, 1):
            nc.gpsimd.partition_broadcast(bcast[:, hs, :], recip[:, hs, :])
        for hs, po in ((0, 0), (1, D)):
            nc.vector.tensor_mul(
                ot_sb[mo][ds(po, D), ts(s, 512)],
                ot_ps[0:D, hs, :],
                bcast[:, hs, :],
            )

    # --- attention block: S^T -> exp -> (mask) -> PV for one (s, j, mo) ---
    def emit_attn(s, j, mo, ot_units, nblk, bg, bg_state):
        off = max(0, j - 4 * s) * P
        n = 512 - off
        s_ps = psum_s.tile([P, 2, 512], F32, name="s_ps")
        for hs, po in ((0, 0), (1, D)):
            nc.tensor.matmul(
                s_ps[:, hs, 0:n],
                kt_sb[mo][ds(po, D), ts(j, P)],
                qt_sb[mo][ds(po, D), ds(512 * s + off, n)],
                start=True,
                stop=True,
            )
        pt = pt_pool.tile([P, 2, 512], BF16)
        nc.scalar.activation(pt[:, :, 0:n], s_ps[:, :, 0:n], Exp, scale=scale)
        # background PE work lands between S^T and PV so the PE chews on it
        # while the exp runs
        pop_bg(bg, bg_state)
        if off > 0 or j == 4 * s:
            # first 128 cols of the ragged region are the diagonal block
            for hs in (0, 1):
                nc.vector.tensor_mul(pt[:, hs, 0:P], pt[:, hs, 0:P], triu[:])
        for hs, h in ((0, 2 * mo), (1, 2 * mo + 1)):
            nc.tensor.matmul(
                ot_units[mo][:, hs, ds(off, n)],
                v_sb[:, j, h, :],
                pt[:, hs, ds(0, n)],
                start=(j == 0),
                stop=(j == nblk - 1),
            )

    # --- background-work pacing: drain bg evenly across a window's slots.
    # Emitted under high_priority so the scheduler slots the (dependency-
    # free) background matmuls into the PE's exp-wait gaps instead of after
    # the exp-dependent PV matmuls. ---
    def pop_bg(bg, bg_state):
        bg_state[0] += 1
        target = (len(bg[1]) * bg_state[0] + bg_state[1] - 1) // bg_state[1]
        while bg[0] < min(target, len(bg[1])):
            with tc.high_priority(offset=48):
                bg[1][bg[0]]()
            bg[0] += 1

    def drain_bg(bg):
        while bg[0] < len(bg[1]):
            bg[1][bg[0]]()
            bg[0] += 1

    def proj_groups_for(tc_):
        groups = []
        for mo in range(M // P):
            for w_sb, dst in ((wq_sb, qt_sb), (wk_sb, kt_sb)):
                groups.append(lambda t=tc_, m=mo, w=w_sb, d=dst: emit_qk_group(t, m, w, d))
        for tt in range(4 * tc_, 4 * tc_ + 4):
            groups.append(lambda t=tt: emit_v_block(t))
        return groups

    # minimal prefix before attention can start: q/k for both head pairs of
    # supertile 0 plus the first V block
    g0 = proj_groups_for(0)
    for g in g0[:4] + [g0[4]]:
        g()
    rest0 = g0[5:]           # v blocks tt1..tt3

    # deferred y-projection schedule: each supertile's 4 tiles become
    # available after its normalization; we push most of them into the last
    # supertile's stream where no projection work remains.
    yproj_carry = []
    carry_cap = {1: 2, 2: 3, 3: 99}

    for s in range(NS):
        nblk = 4 * (s + 1)
        bg_list = []
        if s == 0:
            bg_list += rest0
        if s + 1 < NS:
            bg_list += proj_groups_for(s + 1)
        ncarry = min(carry_cap.get(s, 0), len(yproj_carry))
        for tt in yproj_carry[:ncarry]:
            bg_list.append(lambda t=tt: emit_yproj(t))
        yproj_carry = yproj_carry[ncarry:]

        ot_units = [
            psum_ot.tile([D + 1, 2, 512], F32, name="ot_ps")
            for _ in range(M // P)
        ]

        if s + 1 < NS:
            # mo-inner: both head pairs advance together
            nslots = nblk * 2
            bg = [0, bg_list]
            bg_state = [0, nslots]
            for j in range(nblk):
                for mo in range(M // P):
                    emit_attn(s, j, mo, ot_units, nblk, bg, bg_state)
            drain_bg(bg)
            for mo in range(M // P):
                emit_norm(s, mo, ot_units[mo])
        else:
            # final supertile: mo-outer so head pair 0 finishes (and
            # normalizes) while head pair 1 is still streaming
            half = (len(bg_list) + 1) // 2
            bgA = [0, bg_list[:half]]
            bgB = [0, bg_list[half:]]
            bg_state_a = [0, nblk]
            for j in range(nblk):
                emit_attn(s, j, 0, ot_units, nblk, bgA, bg_state_a)
            drain_bg(bgA)
            emit_norm(s, 0, ot_units[0])
            bg_state_b = [0, nblk]
            for j in range(nblk):
                emit_attn(s, j, 1, ot_units, nblk, bgB, bg_state_b)
            drain_bg(bgB)

            # tail: overlap the mo=1 norm chain with the mo=0 halves of the
            # final output projections, then finish and store
            tts = list(range(4 * s, 4 * s + 4))
            yp = [
                psum_s.tile([P, 2, 512], F32, name="s_ps"),
                psum_s.tile([P, 2, 512], F32, name="s_ps"),
            ]
            for i, tt in enumerate(tts):
                nc.tensor.matmul(
                    yp[i // 2][:, i % 2, :],
                    ot_sb[0][:, ts(tt, P)],
                    wo_sb[:, 0, :],
                    start=True,
                    stop=False,
                )
            emit_norm(s, 1, ot_units[1])
            for i, tt in enumerate(tts):
                nc.tensor.matmul(
                    yp[i // 2][:, i % 2, :],
                    ot_sb[1][:, ts(tt, P)],
                    wo_sb[:, 1, :],
                    start=False,
                    stop=True,
                )
                y_sb = ysb_pool.tile([P, C], BF16)
                evict(y_sb[:], yp[i // 2][:, i % 2, :])
                _ydma_q[i % 2].dma_start(y[ts(tt, P), :], y_sb[:])

        yproj_carry += list(range(4 * s, 4 * s + 4)) if s + 1 < NS else []


def shard_inputs(x, W_qkv, W_out):
    """Full inputs -> list of 8 per-core input dicts (core = b*HG + hg)."""
    bf16 = ml_dtypes.bfloat16
    x = np.asarray(x, dtype=np.float32)
    W_qkv = np.asarray(W_qkv, dtype=np.float32).astype(bf16)
    W_out = np.asarray(W_out, dtype=np.float32).astype(bf16)

    def tile_w(w):  # [C, M] -> [P, KC, M]
        return np.ascontiguousarray(
            w.reshape(KC, P, -1).transpose(1, 0, 2)
        )

    in_maps = []
    for b in range(B):
        xT = np.ascontiguousarray(
            x[b].T.astype(bf16).reshape(KC, P, T)
        )
        for hg in range(HG):
            cols = slice(hg * M, (hg + 1) * M)
            wo_full = W_out[hg * M : (hg + 1) * M, :]     # [M, C]
            in_maps.append(
                {
                    "xT": xT,
                    "wq": tile_w(W_qkv[:, 0 * C :][:, cols]),
                    "wk": tile_w(W_qkv[:, 1 * C :][:, cols]),
                    "wv": tile_w(W_qkv[:, 2 * C :][:, cols]),
                    "wo": np.ascontiguousarray(
                        wo_full.reshape(M // P, P, C).transpose(1, 0, 2)
                    ),
                }
            )
    return in_maps


_NC_CACHE = None


def kernel(x, W_qkv, W_out):
    global _NC_CACHE, _LAST_RESULTS
    if _NC_CACHE is None:
        _NC_CACHE = build_attention_kernel()
    nc = _NC_CACHE
    in_maps = shard_inputs(x, W_qkv, W_out)
    kwargs = {}
    if os.environ.get("BASS_KERNEL_TRACE"):
        kwargs = dict(trace=True, tmpdir=os.environ.get("BASS_KERNEL_TRACE_DIR"))
    res = run_bass_kernel_spmd(nc, in_maps, core_ids=list(range(B * HG)), **kwargs)
    _LAST_RESULTS = res
    out = np.empty((B, T, C), dtype=np.float32)
    for b in range(B):
        out[b] = (
            res.results[b * HG]["y"].astype(np.float32)
            + res.results[b * HG + 1]["y"].astype(np.float32)
        )
    return out
